# revision 6
# baseline (speedup 1.0000x reference)
"""AttentionRouter Trainium2 kernel.

Computes, for packed tokens x [T=32768, H=8, D=128] with B=8 ragged segments
(cu_seq_len [9]), the per-segment mean-pooled features -> tiny MLP router ->
binary mask z [B, H, 1].

Strategy (8 NeuronCores, data-parallel over tokens):
  - Each core owns 4096 tokens (16 MiB of x), streamed f32 over the HWDGE
    queue and consumed by the PE as float32r (same 4-byte data, single-pass
    matmul at moving dim >= 256); PSUM accumulation stays f32.
  - Segment membership masks are built on-device from cu_seq_len via
    compare ops on a host-supplied token-index iota.
  - Partial segment sums (over tokens AND heads) via TensorE mask-matmuls;
    both feature halves accumulate into one PSUM bank so half the head
    reduction is free.
  - A tiny (8x128 f32) AllGather + local sum combines partials across
    cores; segment counts come from cu_seq_len directly (replicated).
  - Every core then (redundantly) runs the 5-layer MLP (bf16 weights via
    gpsimd cast-DMA) on the pooled means and emits z [8, 1]; the host
    takes core 0's output and broadcasts to [B, H, 1].
"""

import sys

if "/opt/trn_rl_repo" not in sys.path:
    sys.path.insert(0, "/opt/trn_rl_repo")

import numpy as np

import concourse.bacc as bacc
import concourse.tile as tile
from concourse import mybir
from concourse.bass_utils import run_bass_kernel_spmd

N_CORES = 8
T, B, H, D = 32768, 8, 8, 128
E = H * D                      # 1024 features per token (heads folded in)
TOK = T // N_CORES             # 4096 tokens per core
NPART = 128
TPB = TOK // NPART             # 32 token-blocks (matmul contraction tiles)
NCHUNK = 8                     # x DMA chunks per core
BPC = TPB // NCHUNK            # 4 token-blocks per DMA chunk

F32 = mybir.dt.float32
BF16 = mybir.dt.bfloat16


def _mlp_dense(nc, pp_mlp, sp, ones_row, a_in, w_sb, b_sb, K, M, act, sim_safe):
    """out[M, 8] = act(W.T @ a_in + b), activations transposed [feat, batch].
    a_in: [128, kch*8] bf16, chunk k at cols [k*8,(k+1)*8). w_sb: [128, kch, M]
    bf16. b_sb: [1, M] bf16. Returns bf16 [128, mch*8]."""
    kch = K // 128
    mch = (M + 127) // 128
    a_out = sp.tile([128, mch * 8], BF16, tag="act")
    for m in range(mch):
        mm = min(128, M - m * 128)
        ps = pp_mlp.tile([128, 8], F32, tag="mlp_ps")
        for k in range(kch):
            nc.tensor.matmul(
                ps[0:mm, :],
                w_sb[:, k, m * 128 : m * 128 + mm],
                a_in[:, k * 8 : (k + 1) * 8],
                start=(k == 0),
                stop=False,
            )
        nc.tensor.matmul(
            ps[0:mm, :],
            b_sb[0:1, m * 128 : m * 128 + mm],
            ones_row[:],
            start=False,
            stop=True,
        )
        if act and not sim_safe:
            # native Silu on ACT (CoreSim lacks it; sim builds use the
            # mathematically identical sigmoid+mult path below)
            nc.scalar.activation(
                a_out[0:mm, m * 8 : (m + 1) * 8], ps[0:mm, :],
                mybir.ActivationFunctionType.Silu,
            )
        elif act:
            sg = sp.tile([128, 8], F32, tag="mlp_sig")
            nc.scalar.activation(
                sg[0:mm, :], ps[0:mm, :], mybir.ActivationFunctionType.Sigmoid
            )
            nc.vector.tensor_tensor(
                a_out[0:mm, m * 8 : (m + 1) * 8], ps[0:mm, :], sg[0:mm, :],
                op=mybir.AluOpType.mult,
            )
        else:
            nc.vector.tensor_copy(a_out[0:mm, m * 8 : (m + 1) * 8], ps[0:mm, :])
    return a_out


def _build_kernel_body(nc, tc, d):
    """d: dict of DRAM tensor handles."""
    import contextlib

    scope = nc.named_scope if hasattr(nc, "named_scope") else (
        lambda name: contextlib.nullcontext()
    )
    with (
        tc.tile_pool(name="xp", bufs=8) as xp,
        tc.tile_pool(name="wp", bufs=1) as wp,
        tc.tile_pool(name="sp", bufs=1) as sp,
        tc.tile_pool(name="spa", bufs=2) as spa,
        tc.tile_pool(name="pp", bufs=1, space="PSUM") as pp,
        tc.tile_pool(name="ppm", bufs=3, space="PSUM") as ppm,
        tc.tile_pool(name="dp", bufs=1, space="DRAM") as dp,
    ):
        # ---- x chunk DMAs issued FIRST so the Sync HWDGE queue starts the
        # big stream immediately; everything small rides gpsimd/SWDGE ----
        F32R = mybir.dt.float32r
        xv = d["x"].ap().rearrange("(p n) e -> p n e", p=128)
        xts = []
        with scope("s_xdma"):
            for c in range(NCHUNK):
                xf = xp.tile([128, BPC, E], F32R, tag="xf", name=f"xf{c}")
                # alternate the two HWDGE rings (SP + ACT) so descriptor gen and
                # completion handling of consecutive chunks pipeline
                eng = nc.sync if c % 2 == 0 else nc.scalar
                eng.dma_start(xf[:], xv[:, c * BPC : (c + 1) * BPC, :])
                xts.append(xf)

        # ---- warm-up collective: a tiny AllGather fired first so the lazy
        # per-execution ncfw/channel setup (~25-40us cold, measured) happens
        # under phase 1; the warmed real gather's machinery is ~8us. The
        # staging DMA must avoid the x-congested HWDGE rings (FIFO per ring)
        # and the trigger's gpsimd DRAIN must precede the slow weight
        # cast-DMAs, hence gpsimd staging + top placement. ----
        wusrc = sp.tile([8, 16], F32)
        nc.vector.memset(wusrc[:], 0.0)
        wuin = dp.tile([8, 16], F32)
        wuout = dp.tile([N_CORES * 8, 16], F32)
        nc.gpsimd.dma_start(wuin[:], wusrc[:])
        nc.gpsimd.collective_compute(
            "AllGather",
            mybir.AluOpType.bypass,
            replica_groups=[list(range(N_CORES))],
            ins=[wuin.opt()],
            outs=[wuout.opt()],
        )

        # ---- small constants / metadata ----
        cu_sb = sp.tile([128, B + 1], F32)
        nc.gpsimd.dma_start(cu_sb[:], d["cu"].ap())
        tidx = sp.tile([128, TPB], F32)
        nc.gpsimd.dma_start(tidx[:], d["tidx"].ap())
        ident = sp.tile([8, 8], F32)
        nc.gpsimd.dma_start(ident[:], d["ident"].ap())

        ones_row = sp.tile([1, 8], BF16)
        nc.vector.memset(ones_row[:], 1.0)

        # ---- segment membership masks from cu_seq_len ----
        # ge[p, j, n] = (token_idx[p, n] >= cu[j]);  mask = ge[:,0:8]-ge[:,1:9]
        ge = sp.tile([128, B + 1, TPB], F32)
        for j in range(B + 1):
            nc.vector.tensor_scalar(
                ge[:, j, :],
                tidx[:],
                cu_sb[:, j : j + 1],
                None,
                op0=mybir.AluOpType.is_ge,
            )
        # mask[p, b, n]: token of (p, n) belongs to segment b (0/1).
        # float32r so the PE runs single-pass; producer must write fp32r
        # (walrus checkMatmultFP32r requires rounded inputs)
        mask = sp.tile([128, B, TPB], mybir.dt.float32r)
        nc.vector.tensor_tensor(
            mask[:], ge[:, 0:B, :], ge[:, 1 : B + 1, :], op=mybir.AluOpType.subtract
        )

        # ---- segment counts from cu (replicated; no collective needed) ----
        counts_row = sp.tile([1, B], F32)
        nc.vector.tensor_tensor(
            counts_row[:], cu_sb[0:1, 1 : B + 1], cu_sb[0:1, 0:B],
            op=mybir.AluOpType.subtract,
        )
        cnt_ps = ppm.tile([B, 1], F32, tag="mlp_ps")
        nc.tensor.matmul(  # transpose [1,B] -> [B,1] via K=1 matmul with ones
            cnt_ps[:], counts_row[:], ident[0:1, 0:1], start=True, stop=True
        )
        # denom = H * max(count, 1)
        denom = sp.tile([B, 1], F32)
        nc.vector.tensor_scalar(
            denom[:], cnt_ps[:], 1.0, float(H),
            op0=mybir.AluOpType.max, op1=mybir.AluOpType.mult,
        )
        recip = sp.tile([B, 1], F32)
        nc.vector.reciprocal(recip[:], denom[:])

        # ---- MLP weights to SBUF as bf16 (gpsimd cast-DMA; overlaps phase 1) ----
        w1_sb = wp.tile([128, 1, 8 * D], BF16)   # W1 [128, 1024]
        nc.gpsimd.dma_start(w1_sb[:], d["w1"].ap().rearrange("(k p) m -> p k m", p=128))
        w2_sb = wp.tile([128, 8, 2 * D], BF16)   # W2 [1024, 256]
        nc.gpsimd.dma_start(w2_sb[:], d["w2"].ap().rearrange("(k p) m -> p k m", p=128))
        w3_sb = wp.tile([128, 2, 4 * D], BF16)   # W3 [256, 512]
        nc.gpsimd.dma_start(w3_sb[:], d["w3"].ap().rearrange("(k p) m -> p k m", p=128))
        w4_sb = wp.tile([128, 4, D], BF16)       # W4 [512, 128]
        nc.gpsimd.dma_start(w4_sb[:], d["w4"].ap().rearrange("(k p) m -> p k m", p=128))
        w5_sb = wp.tile([128, 1, 2], BF16)       # W5 [128, 2]
        nc.gpsimd.dma_start(w5_sb[:], d["w5"].ap().rearrange("(k p) m -> p k m", p=128))
        b_sbs = {}
        for name, n in (("b1", 8 * D), ("b2", 2 * D), ("b3", 4 * D), ("b4", D), ("b5", 2)):
            b_sbs[name] = wp.tile([1, n], BF16, tag=name, name=f"{name}_sb")
            nc.gpsimd.dma_start(b_sbs[name][:], d[name].ap())

        # ---- phase 1: masked segment sums over this core's tokens ----
        # x viewed [128, TPB, E]: partition p, block n holds token p*TPB + n.
        # f32-width DMA at full HWDGE rate; x DRAM + SBUF tiles are declared
        # float32r (same 4-byte data, single-pass PE at moving dim >= 256 —
        # fp32 proper is a 4-pass, and casting to bf16 anywhere is slower).
        # both feature halves accumulate into ONE psum bank: psum[b, h'*128+d]
        # = sum over heads h' and h'+4 — half the head reduction happens for
        # free in the PE accumulator
        ps0 = pp.tile([B, 512], F32)
        with scope("s_stream"):
            for c in range(NCHUNK):
                xf = xts[c]
                for k in range(BPC):
                    n = c * BPC + k
                    first, last = (n == 0), (n == TPB - 1)
                    lhsT = mask[:, :, n]
                    nc.tensor.matmul(ps0[:], lhsT, xf[:, k, 0:512], start=first, stop=False)
                    nc.tensor.matmul(ps0[:], lhsT, xf[:, k, 512:E], start=False, stop=last)

        # ---- finish head-sum: [B, 512] -> [B, 128] ----
        # (tensor_tensor may read at most one input from PSUM, so copy first)
        s512 = sp.tile([B, 512], F32)
        nc.vector.tensor_copy(s512[:], ps0[:])
        s256 = sp.tile([B, 256], F32)
        nc.vector.tensor_tensor(
            s256[:], s512[:, 0:256], s512[:, 256:512], op=mybir.AluOpType.add
        )
        pre = sp.tile([B, D], F32)
        nc.vector.tensor_tensor(
            pre[:], s256[:, 0:128], s256[:, 128:256], op=mybir.AluOpType.add
        )

        # ---- AllGather partial sums across the 8 cores (cheaper ncfw path
        # than AllReduce at this size); sum the 8 partials locally ----
        arin = dp.tile([B, D], F32)
        arout = dp.tile([N_CORES * B, D], F32, addr_space="Shared")
        with scope("s_gather"):
            nc.sync.dma_start(arin[:], pre[:])
            nc.gpsimd.collective_compute(
                "AllGather",
                mybir.AluOpType.bypass,
                replica_groups=[list(range(N_CORES))],
                ins=[arin.opt()],
                outs=[arout.opt()],
            )
            # view gathered [8 cores, 8 segs, 128] as [segs (parts), cores*128]
            post = sp.tile([B, N_CORES, D], F32)
            nc.sync.dma_start(post[:], arout[:].rearrange("(c b) d -> b c d", b=B))
        q512 = sp.tile([B, 4 * D], F32)
        nc.vector.tensor_tensor(
            q512[:], post[:, 0:4, :], post[:, 4:8, :], op=mybir.AluOpType.add
        )
        q256 = sp.tile([B, 2 * D], F32)
        nc.vector.tensor_tensor(
            q256[:], q512[:, 0 : 2 * D], q512[:, 2 * D : 4 * D], op=mybir.AluOpType.add
        )
        # ---- pooled mean: sums / (H * max(count, 1)) fused into final add
        psum_all = sp.tile([B, D], F32)
        nc.vector.tensor_tensor(
            psum_all[:], q256[:, 0:D], q256[:, D : 2 * D], op=mybir.AluOpType.add
        )
        pm = sp.tile([B, D], F32)
        nc.vector.tensor_scalar(
            pm[:], psum_all[:], recip[:], None, op0=mybir.AluOpType.mult
        )

        # ---- transpose pooled mean -> a0 [128, 8] bf16 ----
        pmt = ppm.tile([D, B], F32, tag="mlp_ps")
        nc.tensor.transpose(pmt[:], pm[:], ident[:])
        a0 = sp.tile([D, B], BF16)
        nc.vector.tensor_copy(a0[:], pmt[:])

        # ---- MLP (activations kept transposed: [feature, batch]) ----
        ss = d["sim_safe"]
        with scope("s_mlp"):
            a1 = _mlp_dense(nc, ppm, spa, ones_row, a0, w1_sb, b_sbs["b1"], D, 8 * D, True, ss)
            a2 = _mlp_dense(nc, ppm, spa, ones_row, a1, w2_sb, b_sbs["b2"], 8 * D, 2 * D, False, ss)
            a3 = _mlp_dense(nc, ppm, spa, ones_row, a2, w3_sb, b_sbs["b3"], 2 * D, 4 * D, True, ss)
            a4 = _mlp_dense(nc, ppm, spa, ones_row, a3, w4_sb, b_sbs["b4"], 4 * D, D, True, ss)
            a5 = _mlp_dense(nc, ppm, spa, ones_row, a4, w5_sb, b_sbs["b5"], D, 2, False, ss)

        # ---- logits [2, 8] -> z[b] = (logit1 > logit0) -> out [8, 1] ----
        # a5 is bf16 [2, 8]; transpose needs f32-safe path: cast up via copy
        a5f = sp.tile([2, 8], F32)
        nc.vector.tensor_copy(a5f[:], a5[0:2, 0:8])
        lgt = ppm.tile([B, 2], F32, tag="mlp_ps")
        nc.tensor.transpose(lgt[:], a5f[:], ident[0:2, 0:2])
        lg = sp.tile([B, 2], F32)
        nc.vector.tensor_copy(lg[:], lgt[:])
        z = sp.tile([B, 1], F32)
        nc.vector.tensor_tensor(z[:], lg[:, 1:2], lg[:, 0:1], op=mybir.AluOpType.is_gt)
        nc.sync.dma_start(d["out"].ap(), z[:])


def build(sim_safe=False):
    nc = bacc.Bacc("TRN2", target_bir_lowering=False, debug=False, num_devices=N_CORES)
    d = {"sim_safe": sim_safe}
    d["x"] = nc.dram_tensor("x", [TOK, E], mybir.dt.float32r, kind="ExternalInput")
    d["tidx"] = nc.dram_tensor("tidx", [NPART, TPB], F32, kind="ExternalInput")
    d["cu"] = nc.dram_tensor("cu", [NPART, B + 1], F32, kind="ExternalInput")
    d["ident"] = nc.dram_tensor("ident", [8, 8], F32, kind="ExternalInput")
    d["w1"] = nc.dram_tensor("w1", [D, 8 * D], F32, kind="ExternalInput")
    d["b1"] = nc.dram_tensor("b1", [1, 8 * D], F32, kind="ExternalInput")
    d["w2"] = nc.dram_tensor("w2", [8 * D, 2 * D], F32, kind="ExternalInput")
    d["b2"] = nc.dram_tensor("b2", [1, 2 * D], F32, kind="ExternalInput")
    d["w3"] = nc.dram_tensor("w3", [2 * D, 4 * D], F32, kind="ExternalInput")
    d["b3"] = nc.dram_tensor("b3", [1, 4 * D], F32, kind="ExternalInput")
    d["w4"] = nc.dram_tensor("w4", [4 * D, D], F32, kind="ExternalInput")
    d["b4"] = nc.dram_tensor("b4", [1, D], F32, kind="ExternalInput")
    d["w5"] = nc.dram_tensor("w5", [D, 2], F32, kind="ExternalInput")
    d["b5"] = nc.dram_tensor("b5", [1, 2], F32, kind="ExternalInput")
    d["out"] = nc.dram_tensor("out", [B, 1], F32, kind="ExternalOutput")
    with tile.TileContext(nc) as tc:
        _build_kernel_body(nc, tc, d)
    nc.compile()
    return nc


def make_in_maps(x, cu_seq_len, w1, b1, w2, b2, w3, b3, w4, b4, w5, b5):
    x = np.ascontiguousarray(np.asarray(x, dtype=np.float32)).reshape(T, E)
    cu_f = np.asarray(cu_seq_len, dtype=np.float32)
    cu_rep = np.ascontiguousarray(np.broadcast_to(cu_f, (NPART, B + 1)))
    ident = np.eye(8, dtype=np.float32)
    common = {
        "cu": cu_rep,
        "ident": ident,
        "w1": np.asarray(w1, np.float32), "b1": np.asarray(b1, np.float32).reshape(1, -1),
        "w2": np.asarray(w2, np.float32), "b2": np.asarray(b2, np.float32).reshape(1, -1),
        "w3": np.asarray(w3, np.float32), "b3": np.asarray(b3, np.float32).reshape(1, -1),
        "w4": np.asarray(w4, np.float32), "b4": np.asarray(b4, np.float32).reshape(1, -1),
        "w5": np.asarray(w5, np.float32), "b5": np.asarray(b5, np.float32).reshape(1, -1),
    }
    in_maps = []
    for c in range(N_CORES):
        tidx = (c * TOK + np.arange(TOK, dtype=np.float32)).reshape(NPART, TPB)
        in_maps.append({"x": x[c * TOK : (c + 1) * TOK], "tidx": tidx, **common})
    return in_maps


_NC_CACHE = {}


def _get_nc():
    if "nc" not in _NC_CACHE:
        _NC_CACHE["nc"] = build()
    return _NC_CACHE["nc"]


def kernel(**inputs):
    nc = _get_nc()
    in_maps = make_in_maps(**inputs)
    res = run_bass_kernel_spmd(nc, in_maps, core_ids=list(range(N_CORES)))
    z = np.asarray(res.results[0]["out"], dtype=np.float32).reshape(B, 1, 1)
    return np.ascontiguousarray(np.broadcast_to(z, (B, H, 1)))



# revision 9
# speedup vs baseline: 1.0085x; 1.0085x over previous
"""AttentionRouter Trainium2 kernel.

Computes, for packed tokens x [T=32768, H=8, D=128] with B=8 ragged segments
(cu_seq_len [9]), the per-segment mean-pooled features -> tiny MLP router ->
binary mask z [B, H, 1].

Strategy (8 NeuronCores, data-parallel over tokens):
  - Each core owns 4096 tokens (16 MiB of x), streamed f32 over the two
    HWDGE rings (sync gets 9 of 16 1-MiB chunks, scalar 7 — the scalar/ACT
    ring starts later behind the activation-table loads) and consumed by
    the PE as float32r (same 4-byte data, single-pass matmul at moving
    dim >= 256); PSUM accumulation stays f32.
  - MLP weights are pre-cast to bf16 on the host and loaded through the
    scalar HWDGE ring BEHIND the x chunks: the queue's FIFO drain order
    keeps them out of the x stream's HBM window (they're only needed at
    the very end).
  - Segment membership masks are built on-device from cu_seq_len via
    compare ops on a host-supplied token-index iota.
  - Partial segment sums (over tokens AND head pairs) via TensorE
    mask-matmuls into one PSUM bank [8, 512].
  - One AllReduce(add) [8, 512] combines partials across cores (warmed by
    a same-shape dummy AllReduce fired at kernel start).
  - Head-sum + transpose + 1/(H*count) scaling fused into 4 accumulating
    PE transpose-matmuls against a recip-scaled identity.
  - Every core then (redundantly) runs the 5-layer MLP with biases fused
    into the activation (scalar ACT bias operand) or a vector add; logits
    stay [2, 8] so z = is_gt(row1, row0) needs no final transpose.
"""

import sys

if "/opt/trn_rl_repo" not in sys.path:
    sys.path.insert(0, "/opt/trn_rl_repo")

import numpy as np
import ml_dtypes

import concourse.bacc as bacc
import concourse.tile as tile
from concourse import mybir
from concourse.bass_utils import run_bass_kernel_spmd

N_CORES = 8
T, B, H, D = 32768, 8, 8, 128
E = H * D                      # 1024 features per token (heads folded in)
TOK = T // N_CORES             # 4096 tokens per core
NPART = 128
TPB = TOK // NPART             # 32 token-blocks (matmul contraction tiles)
NCHUNK = 16                    # x DMA chunks per core (1 MiB each)
BPC = TPB // NCHUNK            # 2 token-blocks per DMA chunk
SYNC_CHUNKS = 9                # chunks on the sync HWDGE ring (rest: scalar)

F32 = mybir.dt.float32
BF16 = mybir.dt.bfloat16

# (K, M, act?) per MLP layer
LAYERS = [
    ("1", D, 8 * D, True),
    ("2", 8 * D, 2 * D, False),
    ("3", 2 * D, 4 * D, True),
    ("4", 4 * D, D, True),
    ("5", D, 1, False),   # host-folded: w5[:,1]-w5[:,0]; bias handled via is_gt
]


def _mlp_dense(nc, pp_mlp, sp, a_in, w_sb, bT_sb, K, M, act, sim_safe, out_f32=False):
    """out[M, 8] = act(W.T @ a_in + b), activations transposed [feat, batch].
    a_in: [128, kch*8] bf16, chunk k at cols [k*8,(k+1)*8). w_sb: [128, kch, M]
    bf16. bT_sb: [128, mch] bf16 (bias for m-chunk m in column m). Returns
    [128, mch*8] bf16 (or f32 when out_f32)."""
    kch = K // 128
    mch = (M + 127) // 128
    a_out = sp.tile([128, mch * 8], F32 if out_f32 else BF16, tag="act")
    for m in range(mch):
        mm = min(128, M - m * 128)
        ps = pp_mlp.tile([128, 8], F32, tag="mlp_ps")
        for k in range(kch):
            nc.tensor.matmul(
                ps[0:mm, :],
                w_sb[:, k, m * 128 : m * 128 + mm],
                a_in[:, k * 8 : (k + 1) * 8],
                start=(k == 0),
                stop=(k == kch - 1),
            )
        bias = bT_sb[0:mm, m : m + 1]
        if act and not sim_safe:
            # native Silu with fused bias on ACT (CoreSim lacks Silu; sim
            # builds use the mathematically identical path below)
            nc.scalar.activation(
                a_out[0:mm, m * 8 : (m + 1) * 8], ps[0:mm, :],
                mybir.ActivationFunctionType.Silu, bias=bias,
            )
        elif act:
            pre = sp.tile([128, 8], F32, tag="mlp_pre")
            nc.vector.tensor_scalar(
                pre[0:mm, :], ps[0:mm, :], bias, None, op0=mybir.AluOpType.add
            )
            sg = sp.tile([128, 8], F32, tag="mlp_sig")
            nc.scalar.activation(
                sg[0:mm, :], pre[0:mm, :], mybir.ActivationFunctionType.Sigmoid
            )
            nc.vector.tensor_tensor(
                a_out[0:mm, m * 8 : (m + 1) * 8], pre[0:mm, :], sg[0:mm, :],
                op=mybir.AluOpType.mult,
            )
        else:
            # linear layer: bias add on the (otherwise idle) vector engine
            nc.vector.tensor_scalar(
                a_out[0:mm, m * 8 : (m + 1) * 8], ps[0:mm, :], bias, None,
                op0=mybir.AluOpType.add,
            )
    return a_out


def _build_kernel_body(nc, tc, d):
    """d: dict of DRAM tensor handles."""
    import contextlib

    scope = nc.named_scope if hasattr(nc, "named_scope") else (
        lambda name: contextlib.nullcontext()
    )
    with (
        tc.tile_pool(name="xp", bufs=NCHUNK) as xp,
        tc.tile_pool(name="wp", bufs=1) as wp,
        tc.tile_pool(name="sp", bufs=1) as sp,
        tc.tile_pool(name="spa", bufs=2) as spa,
        tc.tile_pool(name="pp", bufs=1, space="PSUM") as pp,
        tc.tile_pool(name="ppm", bufs=3, space="PSUM") as ppm,
        tc.tile_pool(name="dp", bufs=1, space="DRAM") as dp,
    ):
        # ---- x chunk DMAs issued FIRST so the HWDGE queues start the big
        # stream immediately; small setup rides gpsimd/SWDGE ----
        F32R = mybir.dt.float32r
        xv = d["x"].ap().rearrange("(p n) e -> p n e", p=128)
        xts = []
        with scope("s_xdma"):
            for c in range(NCHUNK):
                xf = xp.tile([128, BPC, E], F32R, tag="xf", name=f"xf{c}")
                eng = nc.sync if c < SYNC_CHUNKS else nc.scalar
                eng.dma_start(xf[:], xv[:, c * BPC : (c + 1) * BPC, :])
                xts.append(xf)

        # ---- MLP weights (bf16, host pre-cast/pre-laid-out) on the scalar
        # HWDGE ring, queued BEHIND its x chunks: FIFO drain order delays
        # the bytes until the x stream is done with the HBM pipe ----
        w_sbs, bT_sbs = {}, {}
        for name, K, M, _ in LAYERS:
            kch, mch = K // 128, (M + 127) // 128
            w_sbs[name] = wp.tile([128, kch, M], BF16, tag=f"w{name}",
                                  name=f"w{name}_sb")
            nc.scalar.dma_start(
                w_sbs[name][:],
                d[f"w{name}"].ap().rearrange("p (k m) -> p k m", k=kch),
            )
            bT_sbs[name] = wp.tile([128, mch], F32, tag=f"b{name}",
                                   name=f"b{name}_sb")
            nc.scalar.dma_start(bT_sbs[name][:], d[f"b{name}"].ap())

        # ---- warm-up collective: a same-shape dummy AllReduce fired first
        # so the lazy per-execution ncfw/channel setup happens under the
        # x stream; the warmed real AllReduce's machinery is cheap. ----
        wusrc = sp.tile([8, 512], F32)
        nc.vector.memset(wusrc[:], 0.0)
        wuin = dp.tile([8, 512], F32)
        wuout = dp.tile([8, 512], F32, addr_space="Shared")
        nc.gpsimd.dma_start(wuin[:], wusrc[:])
        nc.gpsimd.collective_compute(
            "AllReduce",
            mybir.AluOpType.add,
            replica_groups=[list(range(N_CORES))],
            ins=[wuin.opt()],
            outs=[wuout.opt()],
        )

        # ---- small constants / metadata (gpsimd SWDGE; tiny) ----
        cu_sb = sp.tile([128, B + 1], F32)
        nc.gpsimd.dma_start(cu_sb[:], d["cu"].ap())
        tidx = sp.tile([128, TPB], F32)
        nc.gpsimd.dma_start(tidx[:], d["tidx"].ap())
        ident = sp.tile([8, 8], F32)
        nc.gpsimd.dma_start(ident[:], d["ident"].ap())

        # ---- segment membership masks from cu_seq_len ----
        # ge[p, j, n] = (token_idx[p, n] >= cu[j]);  mask = ge[:,0:8]-ge[:,1:9]
        ge = sp.tile([128, B + 1, TPB], F32)
        for j in range(B + 1):
            nc.vector.tensor_scalar(
                ge[:, j, :],
                tidx[:],
                cu_sb[:, j : j + 1],
                None,
                op0=mybir.AluOpType.is_ge,
            )
        # mask[p, b, n]: token of (p, n) belongs to segment b (0/1).
        # float32r so the PE runs single-pass
        mask = sp.tile([128, B, TPB], mybir.dt.float32r)
        nc.vector.tensor_tensor(
            mask[:], ge[:, 0:B, :], ge[:, 1 : B + 1, :], op=mybir.AluOpType.subtract
        )

        # ---- segment counts from cu (replicated; no collective needed) ----
        counts_row = sp.tile([1, B], F32)
        nc.vector.tensor_tensor(
            counts_row[:], cu_sb[0:1, 1 : B + 1], cu_sb[0:1, 0:B],
            op=mybir.AluOpType.subtract,
        )
        cnt_ps = ppm.tile([B, 1], F32, tag="mlp_ps")
        nc.tensor.matmul(  # transpose [1,B] -> [B,1] via K=1 matmul
            cnt_ps[:], counts_row[:], ident[0:1, 0:1], start=True, stop=True
        )
        # denom = H * max(count, 1)
        denom = sp.tile([B, 1], F32)
        nc.vector.tensor_scalar(
            denom[:], cnt_ps[:], 1.0, float(H),
            op0=mybir.AluOpType.max, op1=mybir.AluOpType.mult,
        )
        recip = sp.tile([B, 1], F32)
        nc.vector.reciprocal(recip[:], denom[:])
        # identr[j, b] = I[j, b] * recip[j] — the transpose-matmuls against
        # it fold the mean scaling in for free
        identr = sp.tile([B, B], F32)
        nc.vector.tensor_scalar(
            identr[:], ident[:], recip[:], None, op0=mybir.AluOpType.mult
        )

        # ---- phase 1: masked segment sums over this core's tokens ----
        # x viewed [128, TPB, E]: partition p, block n holds token p*TPB + n.
        # both feature halves accumulate into ONE psum bank: psum[b, h'*128+d]
        # = sum over heads h' and h'+4 — half the head reduction happens for
        # free in the PE accumulator
        ps0 = pp.tile([B, 512], F32)
        with scope("s_stream"):
            for c in range(NCHUNK):
                xf = xts[c]
                for k in range(BPC):
                    n = c * BPC + k
                    first, last = (n == 0), (n == TPB - 1)
                    lhsT = mask[:, :, n]
                    nc.tensor.matmul(ps0[:], lhsT, xf[:, k, 0:512], start=first, stop=False)
                    nc.tensor.matmul(ps0[:], lhsT, xf[:, k, 512:E], start=False, stop=last)

        # ---- AllReduce partial sums [8, 512] across the 8 cores ----
        s512 = sp.tile([B, 512], F32)
        nc.scalar.activation(
            s512[:], ps0[:], mybir.ActivationFunctionType.Copy
        )
        arin = dp.tile([B, 512], F32)
        arout = dp.tile([B, 512], F32, addr_space="Shared")
        with scope("s_gather"):
            nc.gpsimd.dma_start(arin[:], s512[:])
            nc.gpsimd.collective_compute(
                "AllReduce",
                mybir.AluOpType.add,
                replica_groups=[list(range(N_CORES))],
                ins=[arin.opt()],
                outs=[arout.opt()],
            )
            sum512 = sp.tile([B, 512], F32)
            nc.sync.dma_start(sum512[:], arout[:])

        # ---- fused head-sum + transpose + mean scaling: pmt[d, b] =
        # (sum over 4 head-pair chunks)^T scaled by recip via identr ----
        pmt = ppm.tile([D, B], F32, tag="mlp_ps")
        for h in range(4):
            nc.tensor.matmul(
                pmt[:], sum512[:, h * 128 : (h + 1) * 128], identr[:],
                start=(h == 0), stop=(h == 3),
            )
        a0 = sp.tile([D, B], BF16)
        nc.vector.tensor_copy(a0[:], pmt[:])

        # ---- MLP (activations kept transposed: [feature, batch]) ----
        ss = d["sim_safe"]
        with scope("s_mlp"):
            a = a0
            for name, K, M, act in LAYERS[:4]:
                a = _mlp_dense(
                    nc, ppm, spa, a, w_sbs[name], bT_sbs[name], K, M, act, ss,
                )
            # final layer folded to a single logit-difference column:
            # z = (a4 . w5d > -b5d), fused threshold via is_gt scalar
            ps5 = ppm.tile([1, 8], F32, tag="mlp_ps")
            nc.tensor.matmul(
                ps5[:], w_sbs["5"][:, 0, 0:1], a[:, 0:8], start=True, stop=True
            )
            z = sp.tile([1, 8], F32)
            nc.vector.tensor_scalar(
                z[:], ps5[:], bT_sbs["5"][0:1, 0:1], None,
                op0=mybir.AluOpType.is_gt,
            )
        nc.sync.dma_start(d["out"].ap(), z[:])


def build(sim_safe=False):
    nc = bacc.Bacc("TRN2", target_bir_lowering=False, debug=False, num_devices=N_CORES)
    d = {"sim_safe": sim_safe}
    d["x"] = nc.dram_tensor("x", [TOK, E], mybir.dt.float32r, kind="ExternalInput")
    d["tidx"] = nc.dram_tensor("tidx", [NPART, TPB], F32, kind="ExternalInput")
    d["cu"] = nc.dram_tensor("cu", [NPART, B + 1], F32, kind="ExternalInput")
    d["ident"] = nc.dram_tensor("ident", [8, 8], F32, kind="ExternalInput")
    for name, K, M, _ in LAYERS:
        kch, mch = K // 128, (M + 127) // 128
        d[f"w{name}"] = nc.dram_tensor(f"w{name}", [128, kch * M], BF16,
                                       kind="ExternalInput")
        d[f"b{name}"] = nc.dram_tensor(f"b{name}", [128, mch], F32,
                                       kind="ExternalInput")
    d["out"] = nc.dram_tensor("out", [1, B], F32, kind="ExternalOutput")
    with tile.TileContext(nc) as tc:
        _build_kernel_body(nc, tc, d)
    nc.compile()
    return nc


def make_in_maps(x, cu_seq_len, w1, b1, w2, b2, w3, b3, w4, b4, w5, b5):
    x = np.ascontiguousarray(np.asarray(x, dtype=np.float32)).reshape(T, E)
    cu_f = np.asarray(cu_seq_len, dtype=np.float32)
    cu_rep = np.ascontiguousarray(np.broadcast_to(cu_f, (NPART, B + 1)))
    ident = np.eye(8, dtype=np.float32)
    common = {"cu": cu_rep, "ident": ident}
    w5 = np.asarray(w5, np.float32)
    b5 = np.asarray(b5, np.float32).reshape(-1)
    w5d = (w5[:, 1] - w5[:, 0]).reshape(D, 1)
    b5d = np.full((1,), -(b5[1] - b5[0]), np.float32)  # is_gt threshold
    ws = {"1": (w1, b1), "2": (w2, b2), "3": (w3, b3), "4": (w4, b4),
          "5": (w5d, b5d)}
    for name, K, M, _ in LAYERS:
        w, b = ws[name]
        kch, mch = K // 128, (M + 127) // 128
        w = np.asarray(w, np.float32).reshape(kch, 128, M).transpose(1, 0, 2)
        common[f"w{name}"] = np.ascontiguousarray(w.reshape(128, kch * M)).astype(
            ml_dtypes.bfloat16
        )
        bT = np.zeros((128, mch), np.float32)
        bpad = np.zeros(mch * 128, np.float32)
        bpad[:M] = np.asarray(b, np.float32).reshape(-1)
        bT[:, :] = bpad.reshape(mch, 128).T
        common[f"b{name}"] = bT
    in_maps = []
    for c in range(N_CORES):
        tidx = (c * TOK + np.arange(TOK, dtype=np.float32)).reshape(NPART, TPB)
        in_maps.append({"x": x[c * TOK : (c + 1) * TOK], "tidx": tidx, **common})
    return in_maps


_NC_CACHE = {}


def _get_nc():
    if "nc" not in _NC_CACHE:
        _NC_CACHE["nc"] = build()
    return _NC_CACHE["nc"]


def kernel(**inputs):
    nc = _get_nc()
    in_maps = make_in_maps(**inputs)
    res = run_bass_kernel_spmd(nc, in_maps, core_ids=list(range(N_CORES)))
    z = np.asarray(res.results[0]["out"], dtype=np.float32).reshape(B, 1, 1)
    return np.ascontiguousarray(np.broadcast_to(z, (B, H, 1)))
